# revision 4
# baseline (speedup 1.0000x reference)
"""GNN message-passing NodeBlock kernel for 8 Trainium2 NeuronCores.

Problem:
    agg_a = segment_sum(edata_a, conn_a[1], 100000)   # [N, 64]
    agg_b = segment_sum(edata_b, conn_b[1], 100000)   # [N, 64]
    out   = concat([agg_a, agg_b, vdata], 1) @ W + b  # [N, 128]

Sharding: edges are sharded BY RECEIVER RANGE -- core c owns nodes
[c*12544, (c+1)*12544) and receives exactly the edges targeting them, so each
core computes its slice of the aggregation completely locally; no collective.

v2 design (vs the hi-fp8/lo-bf16 baseline at 217us):
  * Edge features travel as an exact fp8 hi/lo split packed per 128-edge tile
    into ONE [128 edges, 128 = hi(64)|lo(64)] stationary operand: 2 B/elem on
    the wire (same as bf16) but a 128-column LDWEIGHTS -> FWL engages, and one
    matmul per tile instead of two.  PSUM rows 0:63 hold the hi partial agg,
    64:127 the lo; the fold (hi+lo) is fused into the phase-2 dense matmuls by
    duplicating W's rows, so it costs nothing.
  * Types a and b accumulate into separate PSUM blocks (full 128 rows each).
  * The one-hot scatter matrices are built on DVE in a transposed layout
    sel[128 slots, 64 cols, n tiles] so that every tensor_tensor operand has a
    16-bit innermost stride-1 AP -> DVE 2x_1p perf mode (2 elem/cyc/lane)
    instead of the 1x broadcast path.  The matmul reads tile t's one-hot as a
    column-strided rhs AP.  The iota comparand is materialized as a constant
    [128, 64*NHP] table so its reads are also stride-1.
  * vdata, W and the output travel in bf16 (host converts); bias stays f32.

SPMD: one program for all 8 cores.  Per-(core,window) tile counts differ, so
windows are sorted by (tiles_a, tiles_b) per core and the per-step tile count
is the max across cores (order statistics align, padding stays small).
Padding slots carry rel=-1 (matches no iota column); their lhsT rows are
whatever the DMA brought (harmless: their one-hot column is all zero).
"""
import numpy as np
import ml_dtypes

import concourse.bass as bass
import concourse.tile as tile
from concourse import mybir
from concourse.bass_utils import run_bass_kernel_spmd
from concourse.vector_clock import ScopedClock

BF16 = ml_dtypes.bfloat16
FP8 = ml_dtypes.float8_e4m3

N_NODES = 100000
N_EDGES = 800000
D_EDGE = 64
D_NODE = 128
D_OUT = 128
N_CORES = 8
WIN = 64                   # nodes per window
WPC = 196                  # windows per core
NPC = WIN * WPC            # nodes per core (12544)
NTOT = NPC * N_CORES       # padded node space (100352)
BLK_STEPS = 8              # windows per phase-2 block (8*64 = 512 cols)
N_BLKS = (WPC + BLK_STEPS - 1) // BLK_STEPS  # 25
OUT_CHUNK = 4              # blocks per outT store

# ---------------------------------------------------------------------------
# compat patches for this container's walrus build
# ---------------------------------------------------------------------------

_MAX_WAITS = 1


def _patched_drain_and_barrier(self, tick_clock, wait_clock):
    nc = self.nc
    probe = nc.sync.nop(nofuse=True, hint="tile_drain_wait0")
    wait_clock.add_sem_waits(
        probe.ins, ScopedClock({None: tick_clock.global_clock})
    )
    si = probe.ins.sync_info
    waits = list(si.on_wait) if si is not None and si.on_wait else []
    if len(waits) > _MAX_WAITS:
        si.on_wait = waits[:_MAX_WAITS]
        for k in range(_MAX_WAITS, len(waits), _MAX_WAITS):
            n = nc.sync.nop(nofuse=True, hint=f"tile_drain_wait{k}")
            n.ins.sync_info = mybir.SyncInfo(
                on_wait=waits[k : k + _MAX_WAITS], on_update=[]
            )
    drain_inst = nc.sync.drain()
    wait_clock.add_sem_waits(
        drain_inst.ins, ScopedClock({None: tick_clock.global_clock})
    )
    dsi = drain_inst.ins.sync_info
    if dsi is not None and dsi.on_wait and len(dsi.on_wait) > _MAX_WAITS:
        dsi.on_wait = []
    nc.all_engine_barrier()
    assert self.sems is not None
    popped = nc._tile_sem_poison_stack.pop()
    assert popped is self._sem_poison
    nc.clear_and_free_semaphores(list(self.sems.allocated().values()))
    nc.all_engine_barrier()


def _split_multi_waits(nc):
    """This walrus build accepts one sync-wait per TPB instruction; move
    extra waits onto preceding same-engine NOPs."""
    for fn in nc.m.functions:
        for blk in fn.blocks:
            out = []
            changed = False
            for inst in blk.instructions:
                si = inst.sync_info
                if si is not None and si.on_wait and len(si.on_wait) > 1:
                    waits = list(si.on_wait)
                    for j, w in enumerate(waits[:-1]):
                        nop = mybir.InstNoOp(
                            name=f"{inst.name}_xw{j}", ins=[], outs=[]
                        )
                        nop.engine = inst.engine
                        nop.sync_info = mybir.SyncInfo(
                            on_wait=[w], on_update=[]
                        )
                        out.append(nop)
                    si.on_wait = [waits[-1]]
                    changed = True
                out.append(inst)
            if changed:
                blk.instructions = out


def _install_ntff_hook_shim():
    import sys
    import types

    if "antenv.axon_hooks" in sys.modules:
        return
    mod = types.ModuleType("antenv.axon_hooks")
    _hook = [None]
    mod.set_axon_ntff_profile_hook = lambda h: _hook.__setitem__(0, h)
    mod.get_axon_ntff_profile_hook = lambda: _hook[0]
    sys.modules["antenv.axon_hooks"] = mod
    try:
        import antenv

        antenv.axon_hooks = mod
    except ImportError:
        pass
    try:
        from trn_agent_boot.trn_boot import _ntff_profile_via_ctypes

        mod.set_axon_ntff_profile_hook(
            _ntff_profile_via_ctypes("/opt/axon/libaxon_pjrt.so")
        )
    except Exception:
        pass


tile.TileContext._drain_and_barrier = _patched_drain_and_barrier
_install_ntff_hook_shim()

# ---------------------------------------------------------------------------
# host-side sharding / packing
# ---------------------------------------------------------------------------


def _schedule(cnt_a2, cnt_b2):
    """Shared-window schedule: per-core window perms + per-step (cross-core
    max) tile counts, block layout, and per-(block,half) rel offsets."""
    ta_all = np.ceil(cnt_a2 / 128).astype(np.int32)
    tb_all = np.ceil(cnt_b2 / 128).astype(np.int32)
    perms = np.argsort(-(ta_all * 100 + tb_all), axis=1, kind="stable")
    tiles_a = np.take_along_axis(ta_all, perms, 1)
    tiles_b = np.take_along_axis(tb_all, perms, 1)
    na_step = np.maximum(tiles_a.max(axis=0), 1).astype(np.int64)  # [WPC]
    nb_step = np.maximum(tiles_b.max(axis=0), 1).astype(np.int64)

    # per-block half sizes (edge tiles, unpadded) and rel offsets (per-half
    # even padding so every rel slice starts 4B-aligned)
    blk_na, blk_nb = [], []
    eoff_a = np.zeros(WPC, np.int64)   # edge-tile offset of step's a-tiles
    eoff_b = np.zeros(WPC, np.int64)
    roff_a = np.zeros(WPC, np.int64)   # rel-slot offset of step's a-tiles
    roff_b = np.zeros(WPC, np.int64)
    blk_e0 = []                        # edge-tile offset of each block
    blk_r_a = []                       # rel offset of block's a half
    blk_r_b = []
    e = 0
    r = 0
    for j in range(N_BLKS):
        i0 = j * BLK_STEPS
        steps = min(BLK_STEPS, WPC - i0)
        na = int(na_step[i0 : i0 + steps].sum())
        nb = int(nb_step[i0 : i0 + steps].sum())
        nap = na + (na & 1)
        nbp = nb + (nb & 1)
        blk_e0.append(e)
        blk_na.append(na)
        blk_nb.append(nb)
        blk_r_a.append(r)
        blk_r_b.append(r + nap)
        o = e
        for i in range(i0, i0 + steps):
            eoff_a[i] = o
            o += na_step[i]
        for i in range(i0, i0 + steps):
            eoff_b[i] = o
            o += nb_step[i]
        o = r
        for i in range(i0, i0 + steps):
            roff_a[i] = o
            o += na_step[i]
        o = r + nap
        for i in range(i0, i0 + steps):
            roff_b[i] = o
            o += nb_step[i]
        e += na + nb
        r += nap + nbp
    T_e = int(e)
    T_r = int(r)
    nhp_max = max(
        max(a + (a & 1) for a in blk_na), max(b + (b & 1) for b in blk_nb)
    )
    return dict(
        perms=perms, na_step=na_step, nb_step=nb_step,
        blk_na=blk_na, blk_nb=blk_nb, blk_e0=blk_e0,
        blk_r_a=blk_r_a, blk_r_b=blk_r_b,
        eoff_a=eoff_a, eoff_b=eoff_b, roff_a=roff_a, roff_b=roff_b,
        T_e=T_e, T_r=T_r, nhp_max=int(nhp_max),
    )


def _preprocess(vdata, edata_a, edata_b, conn_a, conn_b, W_mat, b_vec):
    recv_a = np.asarray(conn_a[1]).astype(np.int64)
    recv_b = np.asarray(conn_b[1]).astype(np.int64)

    def bin_type(recv):
        gwin = recv >> 6  # global 64-node window id (core = gwin // WPC)
        order = np.argsort(gwin, kind="stable")
        counts = np.bincount(gwin, minlength=WPC * N_CORES)
        starts = np.zeros(WPC * N_CORES + 1, dtype=np.int64)
        np.cumsum(counts, out=starts[1:])
        return order, counts.reshape(N_CORES, WPC), starts

    ids_a, cnt_a2, st_a = bin_type(recv_a)
    ids_b, cnt_b2, st_b = bin_type(recv_b)

    S = _schedule(cnt_a2, cnt_b2)
    perms = S["perms"]
    T_e, T_r = S["T_e"], S["T_r"]

    def hilo(e):
        hi = np.asarray(e).astype(FP8)
        lo = (np.asarray(e) - hi.astype(np.float32)).astype(FP8)
        return hi, lo  # [E, 64] each

    h_a, l_a = hilo(edata_a)
    h_b, l_b = hilo(edata_b)

    vdata = np.asarray(vdata)
    vpad = np.zeros((NTOT, D_NODE), dtype=np.float32)
    vpad[:N_NODES] = vdata

    # iota_big[p, c*NHP + t] = c  (constant comparand with stride-1 reads)
    NHP = S["nhp_max"]
    iota = np.ascontiguousarray(
        np.broadcast_to(
            np.repeat(np.arange(WIN, dtype=np.float32), NHP)[None, :],
            (128, WIN * NHP),
        )
    ).astype(BF16)

    Wf = np.asarray(W_mat, dtype=np.float32)
    waD = np.ascontiguousarray(
        np.vstack([Wf[0:64], Wf[0:64]]).astype(BF16)
    )  # [128, 128]
    wbD = np.ascontiguousarray(
        np.vstack([Wf[64:128], Wf[64:128]]).astype(BF16)
    )
    wv = np.ascontiguousarray(Wf[128:256].astype(BF16))
    bf = np.asarray(b_vec).astype(np.float32).reshape(D_OUT, 1)

    in_maps = []
    for c in range(N_CORES):
        slot_eid = np.full(T_e * 128, -1, dtype=np.int64)
        slot_is_a = np.zeros(T_e * 128, dtype=bool)
        rel = np.full(T_r * 128, -1.0, dtype=np.float32)  # [slot, rslot]
        for i in range(WPC):
            w = perms[c][i]
            g = c * WPC + w
            for ids, starts, cnts2, eoff, roff, is_a in (
                (ids_a, st_a, cnt_a2, S["eoff_a"], S["roff_a"], True),
                (ids_b, st_b, cnt_b2, S["eoff_b"], S["roff_b"], False),
            ):
                cnt = cnts2[c, w]
                if cnt == 0:
                    continue
                eids = ids[starts[g] : starts[g] + cnt]
                s0 = eoff[i] * 128
                slot_eid[s0 : s0 + cnt] = eids
                slot_is_a[s0 : s0 + cnt] = is_a
                r0 = roff[i] * 128
                rec = recv_a[eids] if is_a else recv_b[eids]
                rel[r0 : r0 + cnt] = (rec & (WIN - 1)).astype(np.float32)
        idx = np.maximum(slot_eid, 0)
        gh = np.where(slot_is_a[:, None], h_a[idx], h_b[idx])
        gl = np.where(slot_is_a[:, None], l_a[idx], l_b[idx])
        # eh[slot, tile, 0:64]=hi, [64:128]=lo ; pad rows left as-is (their
        # one-hot column is zero)
        eh = np.concatenate([gh, gl], axis=1)  # [T_e*128, 128] fp8
        eh = np.ascontiguousarray(
            eh.reshape(T_e, 128, 128).transpose(1, 0, 2)
        )  # [slot, tile, feat] fp8
        # rel is [slot, rslot] on chip: transpose from [rslot*128] slot-major
        relT = np.ascontiguousarray(
            rel.reshape(T_r, 128).T.astype(BF16)
        )  # [128, T_r]
        base = c * NPC
        nodes = (
            base + (perms[c][:, None] * WIN + np.arange(WIN)[None, :]).reshape(-1)
        )
        vT = np.ascontiguousarray(vpad[nodes].T.astype(BF16))  # [128, NPC]
        in_maps.append(
            {"eh": eh, "rel": relT, "vT": vT, "waD": waD, "wbD": wbD,
             "wv": wv, "bd": bf, "iota": iota}
        )

    sched = (
        tuple(int(x) for x in S["na_step"]),
        tuple(int(x) for x in S["nb_step"]),
    )
    return in_maps, sched, perms


# ---------------------------------------------------------------------------
# device kernel
# ---------------------------------------------------------------------------

_NC_CACHE = {}


def _build(sched):
    na_step, nb_step = sched
    # recompute the block layout directly from the step counts (must match
    # the host-side _schedule)
    na_step = np.asarray(na_step, dtype=np.int64)
    nb_step = np.asarray(nb_step, dtype=np.int64)
    blk_na, blk_nb, blk_e0, blk_r_a, blk_r_b = [], [], [], [], []
    e = r = 0
    for j in range(N_BLKS):
        i0 = j * BLK_STEPS
        steps = min(BLK_STEPS, WPC - i0)
        na = int(na_step[i0 : i0 + steps].sum())
        nb = int(nb_step[i0 : i0 + steps].sum())
        blk_e0.append(e)
        blk_na.append(na)
        blk_nb.append(nb)
        blk_r_a.append(r)
        blk_r_b.append(r + na + (na & 1))
        e += na + nb
        r += na + (na & 1) + nb + (nb & 1)
    T_e, T_r = e, r
    NHP = max(
        max(a + (a & 1) for a in blk_na), max(b + (b & 1) for b in blk_nb)
    )
    max_blk = max(a + b for a, b in zip(blk_na, blk_nb))

    f32 = mybir.dt.float32
    bf16 = mybir.dt.bfloat16
    fp8 = mybir.dt.float8e4

    nc = bass.Bass(trn_type="TRN2")
    eh_d = nc.dram_tensor("eh", [128, T_e, 128], fp8, kind="ExternalInput")
    rel_d = nc.dram_tensor("rel", [128, T_r], bf16, kind="ExternalInput")
    vT_d = nc.dram_tensor("vT", [128, NPC], bf16, kind="ExternalInput")
    waD_d = nc.dram_tensor("waD", [128, D_OUT], bf16, kind="ExternalInput")
    wbD_d = nc.dram_tensor("wbD", [128, D_OUT], bf16, kind="ExternalInput")
    wv_d = nc.dram_tensor("wv", [128, D_OUT], bf16, kind="ExternalInput")
    b_d = nc.dram_tensor("bd", [D_OUT, 1], f32, kind="ExternalInput")
    iota_d = nc.dram_tensor("iota", [128, WIN * NHP], bf16, kind="ExternalInput")
    outT_d = nc.dram_tensor("outT", [128, NPC], bf16, kind="ExternalOutput")

    with tile.TileContext(nc) as tc:
        with (
            tc.tile_pool(name="consts", bufs=1) as cb,
            tc.tile_pool(name="xpool", bufs=3) as x0p,
            tc.tile_pool(name="edges", bufs=3) as ep,
            tc.tile_pool(name="sel", bufs=4) as sp,
            tc.tile_pool(name="out", bufs=2) as op,
            tc.tile_pool(name="psA", bufs=2, space="PSUM") as ppa,
            tc.tile_pool(name="psB", bufs=2, space="PSUM") as ppb,
            tc.tile_pool(name="psO", bufs=2, space="PSUM") as ppo,
        ):
            # consts (scalar/ACT queue); edges stream on the sync/SP queue
            iota_sb = cb.tile([128, WIN * NHP], bf16, tag="iota")
            nc.scalar.dma_start(iota_sb[:], iota_d[:, :])
            rel_sb = cb.tile([128, T_r], bf16, tag="rel")
            nc.scalar.dma_start(rel_sb[:], rel_d[:, :])
            wa_sb = cb.tile([128, D_OUT], bf16, tag="wa")
            nc.scalar.dma_start(wa_sb[:], waD_d[:, :])
            wb_sb = cb.tile([128, D_OUT], bf16, tag="wb")
            nc.scalar.dma_start(wb_sb[:], wbD_d[:, :])
            wv_sb = cb.tile([128, D_OUT], bf16, tag="wv")
            nc.scalar.dma_start(wv_sb[:], wv_d[:, :])
            b_sb = cb.tile([D_OUT, 1], f32, tag="b")
            nc.scalar.dma_start(b_sb[:], b_d[:, :])
            vt_sb = cb.tile([128, NPC], bf16, tag="vt")

            ot = None
            for j in range(N_BLKS):
                i0 = j * BLK_STEPS
                steps = min(BLK_STEPS, WPC - i0)
                cols_blk = steps * WIN
                na_b, nb_b = blk_na[j], blk_nb[j]
                n_blk = na_b + nb_b
                e0 = blk_e0[j]

                et = ep.tile([128, max_blk * 128], fp8, tag="et")
                nc.sync.dma_start(
                    et[:, : n_blk * 128], eh_d[:, e0 : e0 + n_blk, :]
                )
                # vT arrives in 5 chunks woven between the early edge loads
                if j < 5:
                    vc0 = j * (NPC // 5)
                    vc1 = NPC if j == 4 else (j + 1) * (NPC // 5)
                    nc.scalar.dma_start(vt_sb[:, vc0:vc1], vT_d[:, vc0:vc1])

                # one-hot build per half, transposed layout [p, c, t]:
                # every operand innermost stride-1 16-bit -> DVE 2x mode
                sels = []
                for r0, nh in ((blk_r_a[j], na_b), (blk_r_b[j], nb_b)):
                    nhp = nh + (nh & 1)
                    st = sp.tile([128, WIN * NHP], bf16, tag="sel")
                    out_ap = st[:, : WIN * nhp].rearrange(
                        "p (c t) -> p c t", t=nhp
                    )
                    in0 = iota_sb[:].rearrange(
                        "p (c t) -> p c t", t=NHP
                    )[:, :, :nhp]
                    in1 = rel_sb[:, r0 : r0 + nhp].rearrange(
                        "p (one t) -> p one t", one=1
                    ).broadcast_to([128, WIN, nhp])
                    nc.vector.tensor_tensor(
                        out=out_ap, in0=in0, in1=in1,
                        op=mybir.AluOpType.is_equal,
                    )
                    sels.append((st, nhp))

                psa = ppa.tile([128, BLK_STEPS * WIN], f32, tag="pa")
                psb = ppb.tile([128, BLK_STEPS * WIN], f32, tag="pb")
                for half, (n_stp, ps) in enumerate(
                    ((na_step, psa), (nb_step, psb))
                ):
                    st, nhp = sels[half]
                    sel3 = st[:, : WIN * nhp].rearrange(
                        "p (c t) -> p c t", t=nhp
                    )
                    tt = blk_na[j] if half else 0  # block-local tile idx
                    t = 0
                    for stp in range(steps):
                        for k in range(n_stp[i0 + stp]):
                            rhs = sel3[:, :, t : t + 1].rearrange(
                                "p c one -> p (c one)"
                            )
                            nc.tensor.matmul(
                                out=ps[:, stp * WIN : (stp + 1) * WIN],
                                lhsT=et[:, tt * 128 : (tt + 1) * 128],
                                rhs=rhs,
                                start=(k == 0),
                                stop=(k == n_stp[i0 + stp] - 1),
                            )
                            t += 1
                            tt += 1

                xa = x0p.tile([128, BLK_STEPS * WIN], bf16, tag="xa")
                nc.scalar.copy(xa[:, :cols_blk], psa[:, :cols_blk])
                xb = x0p.tile([128, BLK_STEPS * WIN], bf16, tag="xb")
                nc.scalar.copy(xb[:, :cols_blk], psb[:, :cols_blk])

                po = ppo.tile([128, BLK_STEPS * WIN], f32, tag="po")
                nc.tensor.matmul(
                    out=po[:, :cols_blk], lhsT=wa_sb[:], rhs=xa[:, :cols_blk],
                    start=True, stop=False,
                )
                nc.tensor.matmul(
                    out=po[:, :cols_blk], lhsT=wb_sb[:], rhs=xb[:, :cols_blk],
                    start=False, stop=False,
                )
                nc.tensor.matmul(
                    out=po[:, :cols_blk],
                    lhsT=wv_sb[:],
                    rhs=vt_sb[:, i0 * WIN : i0 * WIN + cols_blk],
                    start=False, stop=True,
                )
                jc = j % OUT_CHUNK
                if jc == 0:
                    ot = op.tile(
                        [128, OUT_CHUNK * BLK_STEPS * WIN], bf16, tag="ot"
                    )
                    chunk_col0 = i0 * WIN
                nc.scalar.activation(
                    out=ot[:, jc * BLK_STEPS * WIN : jc * BLK_STEPS * WIN + cols_blk],
                    in_=po[:, :cols_blk],
                    func=mybir.ActivationFunctionType.Identity,
                    bias=b_sb[:, 0:1],
                    scale=1.0,
                )
                if jc == OUT_CHUNK - 1 or j == N_BLKS - 1:
                    chunk_cols = jc * BLK_STEPS * WIN + cols_blk
                    nc.scalar.dma_start(
                        outT_d[:, chunk_col0 : chunk_col0 + chunk_cols],
                        ot[:, :chunk_cols],
                    )
    _split_multi_waits(nc)
    return nc


# ---------------------------------------------------------------------------
# public entry point
# ---------------------------------------------------------------------------


def kernel(vdata, edata_a, edata_b, conn_a, conn_b, W, b, _trace=False):
    in_maps, sched, perms = _preprocess(
        vdata, edata_a, edata_b, conn_a, conn_b, W, b
    )
    nc = _NC_CACHE.get(sched)
    if nc is None:
        nc = _build(sched)
        _NC_CACHE[sched] = nc
    kwargs = {}
    if _trace:
        kwargs = dict(trace=True, trace_cores=[0])
    res = run_bass_kernel_spmd(
        nc, in_maps, core_ids=list(range(N_CORES)), **kwargs
    )

    out_full = np.empty((NTOT, D_OUT), dtype=np.float32)
    for c in range(N_CORES):
        outT = np.asarray(res.results[c]["outT"]).astype(np.float32)
        blocks = outT.reshape(D_OUT, WPC, WIN)
        base = c * NPC
        for i in range(WPC):
            w = perms[c][i]
            out_full[base + w * WIN : base + (w + 1) * WIN] = blocks[:, i, :].T
    out = out_full[:N_NODES]
    if _trace:
        return out, res
    return out


# revision 14
# speedup vs baseline: 1.0304x; 1.0304x over previous
"""GNN message-passing NodeBlock kernel for 8 Trainium2 NeuronCores.

Problem:
    agg_a = segment_sum(edata_a, conn_a[1], 100000)   # [N, 64]
    agg_b = segment_sum(edata_b, conn_b[1], 100000)   # [N, 64]
    out   = concat([agg_a, agg_b, vdata], 1) @ W + b  # [N, 128]

Sharding: edges are sharded BY RECEIVER RANGE -- core c owns nodes
[c*12544, (c+1)*12544) and receives exactly the edges targeting them, so each
core computes its slice of the aggregation completely locally; no collective.

v3 design (vs the hi-fp8/lo-bf16 baseline at 217us):
  * Edge features travel as plain bf16 (2 B/elem, rel err ~2^-9): ONE
    64-column stationary + ONE 64-column matmul per 128-edge tile instead of
    the baseline's two.  (A fp8 hi|lo 128-column-stationary variant measured
    WORSE: with only 64 matmul columns per 128-col LDWEIGHTS the PE array
    duty cycle drops to ~20% and the HAM activity monitor holds the PE at
    its cold 1.2 GHz clock for the whole kernel.)
  * Types a and b accumulate into one PSUM block (feat rows 0:64 / 64:128).
  * The one-hot scatter matrices are built on DVE in a transposed layout
    sel[128 slots, 64 cols, n tiles] so that every tensor_tensor operand has a
    16-bit innermost stride-1 AP -> DVE 2x_1p perf mode (2 elem/cyc/lane)
    instead of the 1x broadcast path.  The matmul reads tile t's one-hot as a
    column-strided rhs AP.  The iota comparand is materialized as a constant
    [128, 64*NHP] table so its reads are also stride-1.
  * vdata, W and the output travel in bf16 (host converts); bias stays f32.

SPMD: one program for all 8 cores.  Per-(core,window) tile counts differ, so
windows are sorted by (tiles_a, tiles_b) per core and the per-step tile count
is the max across cores (order statistics align, padding stays small).
Padding slots carry rel=-1 (matches no iota column); their lhsT rows are
whatever the DMA brought (harmless: their one-hot column is all zero).
"""
import numpy as np
import ml_dtypes

import concourse.bass as bass
import concourse.tile as tile
from concourse import mybir
from concourse.bass_utils import run_bass_kernel_spmd
from concourse.vector_clock import ScopedClock

BF16 = ml_dtypes.bfloat16
FP8 = ml_dtypes.float8_e4m3

N_NODES = 100000
N_EDGES = 800000
D_EDGE = 64
D_NODE = 128
D_OUT = 128
N_CORES = 8
WIN = 64                   # nodes per window
WPC = 196                  # windows per core
NPC = WIN * WPC            # nodes per core (12544)
NTOT = NPC * N_CORES       # padded node space (100352)
BLK_STEPS = 8              # windows per phase-2 block (8*64 = 512 cols)
N_BLKS = (WPC + BLK_STEPS - 1) // BLK_STEPS  # 25
OUT_CHUNK = 4              # blocks per outT store

# ---------------------------------------------------------------------------
# compat patches for this container's walrus build
# ---------------------------------------------------------------------------

_MAX_WAITS = 1


def _patched_drain_and_barrier(self, tick_clock, wait_clock):
    nc = self.nc
    probe = nc.sync.nop(nofuse=True, hint="tile_drain_wait0")
    wait_clock.add_sem_waits(
        probe.ins, ScopedClock({None: tick_clock.global_clock})
    )
    si = probe.ins.sync_info
    waits = list(si.on_wait) if si is not None and si.on_wait else []
    if len(waits) > _MAX_WAITS:
        si.on_wait = waits[:_MAX_WAITS]
        for k in range(_MAX_WAITS, len(waits), _MAX_WAITS):
            n = nc.sync.nop(nofuse=True, hint=f"tile_drain_wait{k}")
            n.ins.sync_info = mybir.SyncInfo(
                on_wait=waits[k : k + _MAX_WAITS], on_update=[]
            )
    drain_inst = nc.sync.drain()
    wait_clock.add_sem_waits(
        drain_inst.ins, ScopedClock({None: tick_clock.global_clock})
    )
    dsi = drain_inst.ins.sync_info
    if dsi is not None and dsi.on_wait and len(dsi.on_wait) > _MAX_WAITS:
        dsi.on_wait = []
    nc.all_engine_barrier()
    assert self.sems is not None
    popped = nc._tile_sem_poison_stack.pop()
    assert popped is self._sem_poison
    nc.clear_and_free_semaphores(list(self.sems.allocated().values()))
    nc.all_engine_barrier()


def _split_multi_waits(nc):
    """This walrus build accepts one sync-wait per TPB instruction; move
    extra waits onto preceding same-engine NOPs."""
    for fn in nc.m.functions:
        for blk in fn.blocks:
            out = []
            changed = False
            for inst in blk.instructions:
                si = inst.sync_info
                if si is not None and si.on_wait and len(si.on_wait) > 1:
                    waits = list(si.on_wait)
                    for j, w in enumerate(waits[:-1]):
                        nop = mybir.InstNoOp(
                            name=f"{inst.name}_xw{j}", ins=[], outs=[]
                        )
                        nop.engine = inst.engine
                        nop.sync_info = mybir.SyncInfo(
                            on_wait=[w], on_update=[]
                        )
                        out.append(nop)
                    si.on_wait = [waits[-1]]
                    changed = True
                out.append(inst)
            if changed:
                blk.instructions = out


def _install_ntff_hook_shim():
    import sys
    import types

    if "antenv.axon_hooks" in sys.modules:
        return
    mod = types.ModuleType("antenv.axon_hooks")
    _hook = [None]
    mod.set_axon_ntff_profile_hook = lambda h: _hook.__setitem__(0, h)
    mod.get_axon_ntff_profile_hook = lambda: _hook[0]
    sys.modules["antenv.axon_hooks"] = mod
    try:
        import antenv

        antenv.axon_hooks = mod
    except ImportError:
        pass
    try:
        from trn_agent_boot.trn_boot import _ntff_profile_via_ctypes

        mod.set_axon_ntff_profile_hook(
            _ntff_profile_via_ctypes("/opt/axon/libaxon_pjrt.so")
        )
    except Exception:
        pass


tile.TileContext._drain_and_barrier = _patched_drain_and_barrier
_install_ntff_hook_shim()

# ---------------------------------------------------------------------------
# host-side sharding / packing
# ---------------------------------------------------------------------------


def _schedule(cnt_a2, cnt_b2):
    """Shared-window schedule: per-core window perms + per-step (cross-core
    max) tile counts, block layout, and per-(block,half) rel offsets."""
    ta_all = np.ceil(cnt_a2 / 128).astype(np.int32)
    tb_all = np.ceil(cnt_b2 / 128).astype(np.int32)
    perms = np.argsort(-(ta_all * 100 + tb_all), axis=1, kind="stable")
    tiles_a = np.take_along_axis(ta_all, perms, 1)
    tiles_b = np.take_along_axis(tb_all, perms, 1)
    na_step = np.maximum(tiles_a.max(axis=0), 1).astype(np.int64)  # [WPC]
    nb_step = np.maximum(tiles_b.max(axis=0), 1).astype(np.int64)

    # per-block half sizes (edge tiles, unpadded) and rel offsets (per-half
    # even padding so every rel slice starts 4B-aligned)
    blk_na, blk_nb = [], []
    eoff_a = np.zeros(WPC, np.int64)   # edge-tile offset of step's a-tiles
    eoff_b = np.zeros(WPC, np.int64)
    roff_a = np.zeros(WPC, np.int64)   # rel-slot offset of step's a-tiles
    roff_b = np.zeros(WPC, np.int64)
    blk_e0 = []                        # edge-tile offset of each block
    blk_r_a = []                       # rel offset of block's a half
    blk_r_b = []
    e = 0
    r = 0
    for j in range(N_BLKS):
        i0 = j * BLK_STEPS
        steps = min(BLK_STEPS, WPC - i0)
        na = int(na_step[i0 : i0 + steps].sum())
        nb = int(nb_step[i0 : i0 + steps].sum())
        nap = na + (na & 1)
        nbp = nb + (nb & 1)
        blk_e0.append(e)
        blk_na.append(na)
        blk_nb.append(nb)
        blk_r_a.append(r)
        blk_r_b.append(r + nap)
        o = e
        for i in range(i0, i0 + steps):
            eoff_a[i] = o
            o += na_step[i]
        for i in range(i0, i0 + steps):
            eoff_b[i] = o
            o += nb_step[i]
        o = r
        for i in range(i0, i0 + steps):
            roff_a[i] = o
            o += na_step[i]
        o = r + nap
        for i in range(i0, i0 + steps):
            roff_b[i] = o
            o += nb_step[i]
        e += na + nb
        r += nap + nbp
    T_e = int(e)
    T_r = int(r)
    nhp_max = max(
        max(a + (a & 1) for a in blk_na), max(b + (b & 1) for b in blk_nb)
    )
    return dict(
        perms=perms, na_step=na_step, nb_step=nb_step,
        blk_na=blk_na, blk_nb=blk_nb, blk_e0=blk_e0,
        blk_r_a=blk_r_a, blk_r_b=blk_r_b,
        eoff_a=eoff_a, eoff_b=eoff_b, roff_a=roff_a, roff_b=roff_b,
        T_e=T_e, T_r=T_r, nhp_max=int(nhp_max),
    )


def _preprocess(vdata, edata_a, edata_b, conn_a, conn_b, W_mat, b_vec):
    recv_a = np.asarray(conn_a[1]).astype(np.int64)
    recv_b = np.asarray(conn_b[1]).astype(np.int64)

    def bin_type(recv):
        gwin = recv >> 6  # global 64-node window id (core = gwin // WPC)
        order = np.argsort(gwin, kind="stable")
        counts = np.bincount(gwin, minlength=WPC * N_CORES)
        starts = np.zeros(WPC * N_CORES + 1, dtype=np.int64)
        np.cumsum(counts, out=starts[1:])
        return order, counts.reshape(N_CORES, WPC), starts

    ids_a, cnt_a2, st_a = bin_type(recv_a)
    ids_b, cnt_b2, st_b = bin_type(recv_b)

    S = _schedule(cnt_a2, cnt_b2)
    perms = S["perms"]
    T_e, T_r = S["T_e"], S["T_r"]

    e_a = np.asarray(edata_a).astype(BF16)
    e_b = np.asarray(edata_b).astype(BF16)

    vdata = np.asarray(vdata)
    vpad = np.zeros((NTOT, D_NODE), dtype=np.float32)
    vpad[:N_NODES] = vdata

    # iota_big[p, c*NHP + t] = c  (constant comparand with stride-1 reads)
    NHP = S["nhp_max"]
    iota = np.ascontiguousarray(
        np.broadcast_to(
            np.repeat(np.arange(WIN, dtype=np.float32), NHP)[None, :],
            (128, WIN * NHP),
        )
    ).astype(BF16)

    Wf = np.asarray(W_mat, dtype=np.float32)
    wab = np.ascontiguousarray(Wf[0:128].astype(BF16))  # [128, 128]
    wv = np.ascontiguousarray(Wf[128:256].astype(BF16))
    bf = np.asarray(b_vec).astype(np.float32).reshape(D_OUT, 1)

    in_maps = []
    for c in range(N_CORES):
        slot_eid = np.full(T_e * 128, -1, dtype=np.int64)
        slot_is_a = np.zeros(T_e * 128, dtype=bool)
        rel = np.full(T_r * 128, -1.0, dtype=np.float32)  # [slot, rslot]
        for i in range(WPC):
            w = perms[c][i]
            g = c * WPC + w
            for ids, starts, cnts2, eoff, roff, is_a in (
                (ids_a, st_a, cnt_a2, S["eoff_a"], S["roff_a"], True),
                (ids_b, st_b, cnt_b2, S["eoff_b"], S["roff_b"], False),
            ):
                cnt = cnts2[c, w]
                if cnt == 0:
                    continue
                eids = ids[starts[g] : starts[g] + cnt]
                s0 = eoff[i] * 128
                slot_eid[s0 : s0 + cnt] = eids
                slot_is_a[s0 : s0 + cnt] = is_a
                r0 = roff[i] * 128
                rec = recv_a[eids] if is_a else recv_b[eids]
                rel[r0 : r0 + cnt] = (rec & (WIN - 1)).astype(np.float32)
        idx = np.maximum(slot_eid, 0)
        eh = np.where(slot_is_a[:, None], e_a[idx], e_b[idx])
        # pad rows left as-is (their one-hot column is zero)
        eh = np.ascontiguousarray(
            eh.reshape(T_e, 128, 64).transpose(1, 0, 2)
        )  # [slot, tile, feat] bf16
        # rel is [slot, rslot] on chip: transpose from [rslot*128] slot-major
        relT = np.ascontiguousarray(
            rel.reshape(T_r, 128).T.astype(BF16)
        )  # [128, T_r]
        base = c * NPC
        nodes = (
            base + (perms[c][:, None] * WIN + np.arange(WIN)[None, :]).reshape(-1)
        )
        vT = np.ascontiguousarray(vpad[nodes].T.astype(BF16))  # [128, NPC]
        in_maps.append(
            {"eh": eh, "rel": relT, "vT": vT, "wab": wab,
             "wv": wv, "bd": bf, "iota": iota}
        )

    sched = (
        tuple(int(x) for x in S["na_step"]),
        tuple(int(x) for x in S["nb_step"]),
    )
    return in_maps, sched, perms


# ---------------------------------------------------------------------------
# device kernel
# ---------------------------------------------------------------------------

_NC_CACHE = {}


def _build(sched):
    na_step, nb_step = sched
    # recompute the block layout directly from the step counts (must match
    # the host-side _schedule)
    na_step = np.asarray(na_step, dtype=np.int64)
    nb_step = np.asarray(nb_step, dtype=np.int64)
    blk_na, blk_nb, blk_e0, blk_r_a, blk_r_b = [], [], [], [], []
    e = r = 0
    for j in range(N_BLKS):
        i0 = j * BLK_STEPS
        steps = min(BLK_STEPS, WPC - i0)
        na = int(na_step[i0 : i0 + steps].sum())
        nb = int(nb_step[i0 : i0 + steps].sum())
        blk_e0.append(e)
        blk_na.append(na)
        blk_nb.append(nb)
        blk_r_a.append(r)
        blk_r_b.append(r + na + (na & 1))
        e += na + nb
        r += na + (na & 1) + nb + (nb & 1)
    T_e, T_r = e, r
    NHP = max(
        max(a + (a & 1) for a in blk_na), max(b + (b & 1) for b in blk_nb)
    )
    max_blk = max(a + b for a, b in zip(blk_na, blk_nb))

    f32 = mybir.dt.float32
    bf16 = mybir.dt.bfloat16

    nc = bass.Bass(trn_type="TRN2")
    eh_d = nc.dram_tensor("eh", [128, T_e, 64], bf16, kind="ExternalInput")
    rel_d = nc.dram_tensor("rel", [128, T_r], bf16, kind="ExternalInput")
    vT_d = nc.dram_tensor("vT", [128, NPC], bf16, kind="ExternalInput")
    wab_d = nc.dram_tensor("wab", [128, D_OUT], bf16, kind="ExternalInput")
    wv_d = nc.dram_tensor("wv", [128, D_OUT], bf16, kind="ExternalInput")
    b_d = nc.dram_tensor("bd", [D_OUT, 1], f32, kind="ExternalInput")
    iota_d = nc.dram_tensor("iota", [128, WIN * NHP], bf16, kind="ExternalInput")
    outT_d = nc.dram_tensor("outT", [128, NPC], bf16, kind="ExternalOutput")

    with tile.TileContext(nc) as tc:
        with (
            tc.tile_pool(name="consts", bufs=1) as cb,
            tc.tile_pool(name="xpool", bufs=3) as x0p,
            tc.tile_pool(name="edges", bufs=3) as ep,
            tc.tile_pool(name="sel", bufs=4) as sp,
            tc.tile_pool(name="out", bufs=2) as op,
            tc.tile_pool(name="psE", bufs=3, space="PSUM") as ppe,
            tc.tile_pool(name="psO", bufs=2, space="PSUM") as ppo,
        ):
            # consts (scalar/ACT queue); edges stream on the sync/SP queue
            iota_sb = cb.tile([128, WIN * NHP], bf16, tag="iota")
            nc.scalar.dma_start(iota_sb[:], iota_d[:, :])
            rel_sb = cb.tile([128, T_r], bf16, tag="rel")
            nc.scalar.dma_start(rel_sb[:], rel_d[:, :])
            wab_sb = cb.tile([128, D_OUT], bf16, tag="wab")
            nc.scalar.dma_start(wab_sb[:], wab_d[:, :])
            wv_sb = cb.tile([128, D_OUT], bf16, tag="wv")
            nc.scalar.dma_start(wv_sb[:], wv_d[:, :])
            b_sb = cb.tile([D_OUT, 1], f32, tag="b")
            nc.scalar.dma_start(b_sb[:], b_d[:, :])
            vt_sb = cb.tile([128, NPC], bf16, tag="vt")

            ot = None
            for j in range(N_BLKS):
                i0 = j * BLK_STEPS
                steps = min(BLK_STEPS, WPC - i0)
                cols_blk = steps * WIN
                na_b, nb_b = blk_na[j], blk_nb[j]
                n_blk = na_b + nb_b
                e0 = blk_e0[j]

                et = ep.tile([128, max_blk * 64], bf16, tag="et")
                nc.sync.dma_start(
                    et[:, : n_blk * 64], eh_d[:, e0 : e0 + n_blk, :]
                )
                # vT arrives in 5 chunks woven between the early edge loads
                if j < 5:
                    vc0 = j * (NPC // 5)
                    vc1 = NPC if j == 4 else (j + 1) * (NPC // 5)
                    nc.scalar.dma_start(vt_sb[:, vc0:vc1], vT_d[:, vc0:vc1])

                # one-hot build per half, transposed layout [p, c, t]:
                # every operand innermost stride-1 16-bit -> DVE 2x mode
                sels = []
                for r0, nh in ((blk_r_a[j], na_b), (blk_r_b[j], nb_b)):
                    nhp = nh + (nh & 1)
                    st = sp.tile([128, WIN * NHP], bf16, tag="sel")
                    out_ap = st[:, : WIN * nhp].rearrange(
                        "p (c t) -> p c t", t=nhp
                    )
                    in0 = iota_sb[:].rearrange(
                        "p (c t) -> p c t", t=NHP
                    )[:, :, :nhp]
                    in1 = rel_sb[:, r0 : r0 + nhp].rearrange(
                        "p (one t) -> p one t", one=1
                    ).broadcast_to([128, WIN, nhp])
                    nc.vector.tensor_tensor(
                        out=out_ap, in0=in0, in1=in1,
                        op=mybir.AluOpType.is_equal,
                    )
                    sels.append((st, nhp))

                ps = ppe.tile([128, BLK_STEPS * WIN], f32, tag="ps")
                for half, n_stp in enumerate((na_step, nb_step)):
                    r0 = half * 64  # type a -> feat rows 0:64, b -> 64:128
                    st, nhp = sels[half]
                    sel3 = st[:, : WIN * nhp].rearrange(
                        "p (c t) -> p c t", t=nhp
                    )
                    tt = blk_na[j] if half else 0  # block-local tile idx
                    t = 0
                    for stp in range(steps):
                        for k in range(n_stp[i0 + stp]):
                            rhs = sel3[:, :, t : t + 1].rearrange(
                                "p c one -> p (c one)"
                            )
                            nc.tensor.matmul(
                                out=ps[
                                    r0 : r0 + 64,
                                    stp * WIN : (stp + 1) * WIN,
                                ],
                                lhsT=et[:, tt * 64 : (tt + 1) * 64],
                                rhs=rhs,
                                start=(k == 0),
                                stop=(k == n_stp[i0 + stp] - 1),
                            )
                            t += 1
                            tt += 1

                x0 = x0p.tile([128, BLK_STEPS * WIN], bf16, tag="x0")
                nc.scalar.copy(x0[:, :cols_blk], ps[:, :cols_blk])

                po = ppo.tile([128, BLK_STEPS * WIN], f32, tag="po")
                nc.tensor.matmul(
                    out=po[:, :cols_blk], lhsT=wab_sb[:], rhs=x0[:, :cols_blk],
                    start=True, stop=False,
                )
                nc.tensor.matmul(
                    out=po[:, :cols_blk],
                    lhsT=wv_sb[:],
                    rhs=vt_sb[:, i0 * WIN : i0 * WIN + cols_blk],
                    start=False, stop=True,
                )
                jc = j % OUT_CHUNK
                if jc == 0:
                    ot = op.tile(
                        [128, OUT_CHUNK * BLK_STEPS * WIN], bf16, tag="ot"
                    )
                    chunk_col0 = i0 * WIN
                nc.scalar.activation(
                    out=ot[:, jc * BLK_STEPS * WIN : jc * BLK_STEPS * WIN + cols_blk],
                    in_=po[:, :cols_blk],
                    func=mybir.ActivationFunctionType.Identity,
                    bias=b_sb[:, 0:1],
                    scale=1.0,
                )
                if jc == OUT_CHUNK - 1 or j == N_BLKS - 1:
                    chunk_cols = jc * BLK_STEPS * WIN + cols_blk
                    nc.scalar.dma_start(
                        outT_d[:, chunk_col0 : chunk_col0 + chunk_cols],
                        ot[:, :chunk_cols],
                    )
    _split_multi_waits(nc)
    return nc


# ---------------------------------------------------------------------------
# public entry point
# ---------------------------------------------------------------------------


def kernel(vdata, edata_a, edata_b, conn_a, conn_b, W, b, _trace=False):
    in_maps, sched, perms = _preprocess(
        vdata, edata_a, edata_b, conn_a, conn_b, W, b
    )
    nc = _NC_CACHE.get(sched)
    if nc is None:
        nc = _build(sched)
        _NC_CACHE[sched] = nc
    kwargs = {}
    if _trace:
        kwargs = dict(trace=True, trace_cores=[0])
    res = run_bass_kernel_spmd(
        nc, in_maps, core_ids=list(range(N_CORES)), **kwargs
    )

    out_full = np.empty((NTOT, D_OUT), dtype=np.float32)
    for c in range(N_CORES):
        outT = np.asarray(res.results[c]["outT"]).astype(np.float32)
        blocks = outT.reshape(D_OUT, WPC, WIN)
        base = c * NPC
        for i in range(WPC):
            w = perms[c][i]
            out_full[base + w * WIN : base + (w + 1) * WIN] = blocks[:, i, :].T
    out = out_full[:N_NODES]
    if _trace:
        return out, res
    return out


# revision 25
# speedup vs baseline: 2.1949x; 2.1302x over previous
"""GNN message-passing NodeBlock kernel for 8 Trainium2 NeuronCores.

Problem:
    agg_a = segment_sum(edata_a, conn_a[1], 100000)   # [N, 64]
    agg_b = segment_sum(edata_b, conn_b[1], 100000)   # [N, 64]
    out   = concat([agg_a, agg_b, vdata], 1) @ W + b  # [N, 128]

Sharding: edges are sharded BY RECEIVER RANGE -- core c owns nodes
[c*12544, (c+1)*12544) and receives exactly the edges targeting them, so each
core computes its slice of the aggregation completely locally; no collective.

v3 design (vs the hi-fp8/lo-bf16 baseline at 217us):
  * Edge features travel as plain bf16 (2 B/elem, rel err ~2^-9): ONE
    64-column stationary + ONE 64-column matmul per 128-edge tile instead of
    the baseline's two.  (A fp8 hi|lo 128-column-stationary variant measured
    WORSE: with only 64 matmul columns per 128-col LDWEIGHTS the PE array
    duty cycle drops to ~20% and the HAM activity monitor holds the PE at
    its cold 1.2 GHz clock for the whole kernel.)
  * Types a and b accumulate into one PSUM block (feat rows 0:64 / 64:128).
  * The one-hot scatter matrices are built on DVE in the tile-major layout
    (contiguous matmul rhs -- a column-strided rhs AP measured ~133ns/MM).
    To still hit the DVE 2x_1p perf mode (2 elem/cyc/lane; needs a 16-bit
    innermost stride-1 AP on EVERY operand, where plain broadcast APs fall
    to 1x), the rel comparand is host-duplicated into adjacent pairs
    (relx[2t]=relx[2t+1]=rel[t]) and read via a 4D AP with innermost [1,2],
    and the iota comparand is materialized tile-major as a constant
    [128, NHP*64] table so its reads collapse to contiguous.
  * vdata, W and the output travel in bf16 (host converts); bias stays f32.

SPMD: one program for all 8 cores.  Per-(core,window) tile counts differ, so
windows are sorted by (tiles_a, tiles_b) per core and the per-step tile count
is the max across cores (order statistics align, padding stays small).
Padding slots carry rel=-1 (matches no iota column); their lhsT rows are
whatever the DMA brought (harmless: their one-hot column is all zero).
"""
import numpy as np
import ml_dtypes

import concourse.bass as bass
import concourse.tile as tile
from concourse import mybir
from concourse.bass_utils import run_bass_kernel_spmd
from concourse.vector_clock import ScopedClock

BF16 = ml_dtypes.bfloat16
FP8 = ml_dtypes.float8_e4m3

N_NODES = 100000
N_EDGES = 800000
D_EDGE = 64
D_NODE = 128
D_OUT = 128
N_CORES = 8
WIN = 64                   # nodes per window
WPC = 196                  # windows per core
NPC = WIN * WPC            # nodes per core (12544)
NTOT = NPC * N_CORES       # padded node space (100352)
BLK_STEPS = 8              # windows per phase-2 block (8*64 = 512 cols)
N_BLKS = (WPC + BLK_STEPS - 1) // BLK_STEPS  # 25
OUT_CHUNK = 4              # blocks per outT store

# ---------------------------------------------------------------------------
# compat patches for this container's walrus build
# ---------------------------------------------------------------------------

_MAX_WAITS = 1


def _patched_drain_and_barrier(self, tick_clock, wait_clock):
    nc = self.nc
    probe = nc.sync.nop(nofuse=True, hint="tile_drain_wait0")
    wait_clock.add_sem_waits(
        probe.ins, ScopedClock({None: tick_clock.global_clock})
    )
    si = probe.ins.sync_info
    waits = list(si.on_wait) if si is not None and si.on_wait else []
    if len(waits) > _MAX_WAITS:
        si.on_wait = waits[:_MAX_WAITS]
        for k in range(_MAX_WAITS, len(waits), _MAX_WAITS):
            n = nc.sync.nop(nofuse=True, hint=f"tile_drain_wait{k}")
            n.ins.sync_info = mybir.SyncInfo(
                on_wait=waits[k : k + _MAX_WAITS], on_update=[]
            )
    drain_inst = nc.sync.drain()
    wait_clock.add_sem_waits(
        drain_inst.ins, ScopedClock({None: tick_clock.global_clock})
    )
    dsi = drain_inst.ins.sync_info
    if dsi is not None and dsi.on_wait and len(dsi.on_wait) > _MAX_WAITS:
        dsi.on_wait = []
    nc.all_engine_barrier()
    assert self.sems is not None
    popped = nc._tile_sem_poison_stack.pop()
    assert popped is self._sem_poison
    nc.clear_and_free_semaphores(list(self.sems.allocated().values()))
    nc.all_engine_barrier()


def _split_multi_waits(nc):
    """This walrus build accepts one sync-wait per TPB instruction; move
    extra waits onto preceding same-engine NOPs."""
    for fn in nc.m.functions:
        for blk in fn.blocks:
            out = []
            changed = False
            for inst in blk.instructions:
                si = inst.sync_info
                if si is not None and si.on_wait and len(si.on_wait) > 1:
                    waits = list(si.on_wait)
                    for j, w in enumerate(waits[:-1]):
                        nop = mybir.InstNoOp(
                            name=f"{inst.name}_xw{j}", ins=[], outs=[]
                        )
                        nop.engine = inst.engine
                        nop.sync_info = mybir.SyncInfo(
                            on_wait=[w], on_update=[]
                        )
                        out.append(nop)
                    si.on_wait = [waits[-1]]
                    changed = True
                out.append(inst)
            if changed:
                blk.instructions = out


def _install_ntff_hook_shim():
    import sys
    import types

    if "antenv.axon_hooks" in sys.modules:
        return
    mod = types.ModuleType("antenv.axon_hooks")
    _hook = [None]
    mod.set_axon_ntff_profile_hook = lambda h: _hook.__setitem__(0, h)
    mod.get_axon_ntff_profile_hook = lambda: _hook[0]
    sys.modules["antenv.axon_hooks"] = mod
    try:
        import antenv

        antenv.axon_hooks = mod
    except ImportError:
        pass
    try:
        from trn_agent_boot.trn_boot import _ntff_profile_via_ctypes

        mod.set_axon_ntff_profile_hook(
            _ntff_profile_via_ctypes("/opt/axon/libaxon_pjrt.so")
        )
    except Exception:
        pass


tile.TileContext._drain_and_barrier = _patched_drain_and_barrier
_install_ntff_hook_shim()

# ---------------------------------------------------------------------------
# host-side sharding / packing
# ---------------------------------------------------------------------------


def _schedule(cnt_a2, cnt_b2):
    """Shared-window schedule: per-core window perms + per-step (cross-core
    max) tile counts, block layout, and per-(block,half) rel offsets."""
    ta_all = np.ceil(cnt_a2 / 128).astype(np.int32)
    tb_all = np.ceil(cnt_b2 / 128).astype(np.int32)
    perms = np.argsort(-(ta_all * 100 + tb_all), axis=1, kind="stable")
    tiles_a = np.take_along_axis(ta_all, perms, 1)
    tiles_b = np.take_along_axis(tb_all, perms, 1)
    na_step = np.maximum(tiles_a.max(axis=0), 1).astype(np.int64)  # [WPC]
    nb_step = np.maximum(tiles_b.max(axis=0), 1).astype(np.int64)

    # per-block half sizes (edge tiles); rel slots == edge slots
    blk_na, blk_nb = [], []
    eoff_a = np.zeros(WPC, np.int64)   # edge-tile offset of step's a-tiles
    eoff_b = np.zeros(WPC, np.int64)
    blk_e0 = []                        # edge-tile offset of each block
    e = 0
    for j in range(N_BLKS):
        i0 = j * BLK_STEPS
        steps = min(BLK_STEPS, WPC - i0)
        na = int(na_step[i0 : i0 + steps].sum())
        nb = int(nb_step[i0 : i0 + steps].sum())
        blk_e0.append(e)
        blk_na.append(na)
        blk_nb.append(nb)
        o = e
        for i in range(i0, i0 + steps):
            eoff_a[i] = o
            o += na_step[i]
        for i in range(i0, i0 + steps):
            eoff_b[i] = o
            o += nb_step[i]
        e += na + nb
    T_e = int(e)
    nhp_max = max(max(blk_na), max(blk_nb))
    return dict(
        perms=perms, na_step=na_step, nb_step=nb_step,
        blk_na=blk_na, blk_nb=blk_nb, blk_e0=blk_e0,
        eoff_a=eoff_a, eoff_b=eoff_b,
        T_e=T_e, nhp_max=int(nhp_max),
    )


def _preprocess(vdata, edata_a, edata_b, conn_a, conn_b, W_mat, b_vec):
    recv_a = np.asarray(conn_a[1]).astype(np.int64)
    recv_b = np.asarray(conn_b[1]).astype(np.int64)

    def bin_type(recv):
        gwin = recv >> 6  # global 64-node window id (core = gwin // WPC)
        order = np.argsort(gwin, kind="stable")
        counts = np.bincount(gwin, minlength=WPC * N_CORES)
        starts = np.zeros(WPC * N_CORES + 1, dtype=np.int64)
        np.cumsum(counts, out=starts[1:])
        return order, counts.reshape(N_CORES, WPC), starts

    ids_a, cnt_a2, st_a = bin_type(recv_a)
    ids_b, cnt_b2, st_b = bin_type(recv_b)

    S = _schedule(cnt_a2, cnt_b2)
    perms = S["perms"]
    T_e = S["T_e"]

    e_a = np.asarray(edata_a).astype(BF16)
    e_b = np.asarray(edata_b).astype(BF16)

    vdata = np.asarray(vdata)
    vpad = np.zeros((NTOT, D_NODE), dtype=np.float32)
    vpad[:N_NODES] = vdata

    # iota_full[p, t*64 + c] = c  (tile-major constant comparand)
    NHP = S["nhp_max"]
    iota = np.ascontiguousarray(
        np.broadcast_to(
            np.tile(np.arange(WIN, dtype=np.float32), NHP)[None, :],
            (128, WIN * NHP),
        )
    ).astype(BF16)

    Wf = np.asarray(W_mat, dtype=np.float32)
    wab = np.ascontiguousarray(Wf[0:128].astype(BF16))  # [128, 128]
    wv = np.ascontiguousarray(Wf[128:256].astype(BF16))
    bf = np.asarray(b_vec).astype(np.float32).reshape(D_OUT, 1)

    in_maps = []
    for c in range(N_CORES):
        slot_eid = np.full(T_e * 128, -1, dtype=np.int64)
        slot_is_a = np.zeros(T_e * 128, dtype=bool)
        rel = np.full(T_e * 128, -1.0, dtype=np.float32)  # [tile*128 slots]
        for i in range(WPC):
            w = perms[c][i]
            g = c * WPC + w
            for ids, starts, cnts2, eoff, is_a in (
                (ids_a, st_a, cnt_a2, S["eoff_a"], True),
                (ids_b, st_b, cnt_b2, S["eoff_b"], False),
            ):
                cnt = cnts2[c, w]
                if cnt == 0:
                    continue
                eids = ids[starts[g] : starts[g] + cnt]
                s0 = eoff[i] * 128
                slot_eid[s0 : s0 + cnt] = eids
                slot_is_a[s0 : s0 + cnt] = is_a
                rec = recv_a[eids] if is_a else recv_b[eids]
                rel[s0 : s0 + cnt] = (rec & (WIN - 1)).astype(np.float32)
        idx = np.maximum(slot_eid, 0)
        eh = np.where(slot_is_a[:, None], e_a[idx], e_b[idx])
        # pad rows left as-is (their one-hot column is zero)
        eh = np.ascontiguousarray(
            eh.reshape(T_e, 128, 64).transpose(1, 0, 2)
        )  # [slot, tile, feat] bf16
        # relx[p, 2t] = relx[p, 2t+1] = rel[p, t]  (pairs for DVE 2x packing)
        relT = rel.reshape(T_e, 128).T.astype(BF16)  # [128, T_e]
        relx = np.ascontiguousarray(
            np.repeat(relT, 2, axis=1)
        )  # [128, 2*T_e]
        base = c * NPC
        nodes = (
            base + (perms[c][:, None] * WIN + np.arange(WIN)[None, :]).reshape(-1)
        )
        vT = np.ascontiguousarray(vpad[nodes].T.astype(BF16))  # [128, NPC]
        in_maps.append(
            {"eh": eh, "rel": relx, "vT": vT, "wab": wab,
             "wv": wv, "bd": bf, "iota": iota}
        )

    sched = (
        tuple(int(x) for x in S["na_step"]),
        tuple(int(x) for x in S["nb_step"]),
    )
    return in_maps, sched, perms


# ---------------------------------------------------------------------------
# device kernel
# ---------------------------------------------------------------------------

_NC_CACHE = {}


def _build(sched):
    na_step, nb_step = sched
    # recompute the block layout directly from the step counts (must match
    # the host-side _schedule)
    na_step = np.asarray(na_step, dtype=np.int64)
    nb_step = np.asarray(nb_step, dtype=np.int64)
    blk_na, blk_nb, blk_e0 = [], [], []
    e = 0
    for j in range(N_BLKS):
        i0 = j * BLK_STEPS
        steps = min(BLK_STEPS, WPC - i0)
        na = int(na_step[i0 : i0 + steps].sum())
        nb = int(nb_step[i0 : i0 + steps].sum())
        blk_e0.append(e)
        blk_na.append(na)
        blk_nb.append(nb)
        e += na + nb
    T_e = e
    NHP = max(max(blk_na), max(blk_nb))
    max_blk = max(a + b for a, b in zip(blk_na, blk_nb))

    f32 = mybir.dt.float32
    bf16 = mybir.dt.bfloat16

    nc = bass.Bass(trn_type="TRN2")
    eh_d = nc.dram_tensor("eh", [128, T_e, 64], bf16, kind="ExternalInput")
    rel_d = nc.dram_tensor("rel", [128, 2 * T_e], bf16, kind="ExternalInput")
    vT_d = nc.dram_tensor("vT", [128, NPC], bf16, kind="ExternalInput")
    wab_d = nc.dram_tensor("wab", [128, D_OUT], bf16, kind="ExternalInput")
    wv_d = nc.dram_tensor("wv", [128, D_OUT], bf16, kind="ExternalInput")
    b_d = nc.dram_tensor("bd", [D_OUT, 1], f32, kind="ExternalInput")
    iota_d = nc.dram_tensor("iota", [128, WIN * NHP], bf16, kind="ExternalInput")
    outT_d = nc.dram_tensor("outT", [128, NPC], bf16, kind="ExternalOutput")

    with tile.TileContext(nc) as tc:
        with (
            tc.tile_pool(name="consts", bufs=1) as cb,
            tc.tile_pool(name="xpool", bufs=3) as x0p,
            tc.tile_pool(name="edges", bufs=3) as ep,
            tc.tile_pool(name="sel", bufs=4) as sp,
            tc.tile_pool(name="out", bufs=2) as op,
            tc.tile_pool(name="psE", bufs=3, space="PSUM") as ppe,
            tc.tile_pool(name="psO", bufs=2, space="PSUM") as ppo,
        ):
            # consts (scalar/ACT queue); edges stream on the sync/SP queue
            iota_sb = cb.tile([128, WIN * NHP], bf16, tag="iota")
            nc.scalar.dma_start(iota_sb[:], iota_d[:, :])
            rel_sb = cb.tile([128, 2 * T_e], bf16, tag="rel")
            nc.scalar.dma_start(rel_sb[:], rel_d[:, :])
            wab_sb = cb.tile([128, D_OUT], bf16, tag="wab")
            nc.scalar.dma_start(wab_sb[:], wab_d[:, :])
            wv_sb = cb.tile([128, D_OUT], bf16, tag="wv")
            nc.scalar.dma_start(wv_sb[:], wv_d[:, :])
            b_sb = cb.tile([D_OUT, 1], f32, tag="b")
            nc.scalar.dma_start(b_sb[:], b_d[:, :])
            vt_sb = cb.tile([128, NPC], bf16, tag="vt")

            ot = None
            for j in range(N_BLKS):
                i0 = j * BLK_STEPS
                steps = min(BLK_STEPS, WPC - i0)
                cols_blk = steps * WIN
                na_b, nb_b = blk_na[j], blk_nb[j]
                n_blk = na_b + nb_b
                e0 = blk_e0[j]

                et = ep.tile([128, max_blk * 64], bf16, tag="et")
                nc.sync.dma_start(
                    et[:, : n_blk * 64], eh_d[:, e0 : e0 + n_blk, :]
                )
                # vT arrives in 5 chunks woven between the early edge loads
                if j < 5:
                    vc0 = j * (NPC // 5)
                    vc1 = NPC if j == 4 else (j + 1) * (NPC // 5)
                    nc.scalar.dma_start(vt_sb[:, vc0:vc1], vT_d[:, vc0:vc1])

                # one-hot build per half, tile-major layout: contiguous out
                # and iota; relx pairs via 4D AP with innermost [1,2] ->
                # every operand 16-bit innermost stride-1 -> DVE 2x mode
                sels = []
                for r0, nh in ((e0, na_b), (e0 + na_b, nb_b)):
                    st = sp.tile([128, WIN * NHP], bf16, tag="sel")
                    in1 = rel_sb[:, 2 * r0 : 2 * (r0 + nh)].rearrange(
                        "p (t one cj) -> p t one cj", one=1, cj=2
                    ).broadcast_to([128, nh, WIN // 2, 2])
                    nc.vector.tensor_tensor(
                        out=st[:, : WIN * nh].rearrange(
                            "p (t ci cj) -> p t ci cj", ci=WIN // 2, cj=2
                        ),
                        in0=iota_sb[:, : WIN * nh].rearrange(
                            "p (t ci cj) -> p t ci cj", ci=WIN // 2, cj=2
                        ),
                        in1=in1,
                        op=mybir.AluOpType.is_equal,
                    )
                    sels.append(st)

                ps = ppe.tile([128, BLK_STEPS * WIN], f32, tag="ps")
                for half, n_stp in enumerate((na_step, nb_step)):
                    r0 = half * 64  # type a -> feat rows 0:64, b -> 64:128
                    st = sels[half]
                    tt = blk_na[j] if half else 0  # block-local tile idx
                    t = 0
                    for stp in range(steps):
                        for k in range(n_stp[i0 + stp]):
                            nc.tensor.matmul(
                                out=ps[
                                    r0 : r0 + 64,
                                    stp * WIN : (stp + 1) * WIN,
                                ],
                                lhsT=et[:, tt * 64 : (tt + 1) * 64],
                                rhs=st[:, t * WIN : (t + 1) * WIN],
                                start=(k == 0),
                                stop=(k == n_stp[i0 + stp] - 1),
                            )
                            t += 1
                            tt += 1

                x0 = x0p.tile([128, BLK_STEPS * WIN], bf16, tag="x0")
                nc.scalar.copy(x0[:, :cols_blk], ps[:, :cols_blk])

                po = ppo.tile([128, BLK_STEPS * WIN], f32, tag="po")
                nc.tensor.matmul(
                    out=po[:, :cols_blk], lhsT=wab_sb[:], rhs=x0[:, :cols_blk],
                    start=True, stop=False,
                )
                nc.tensor.matmul(
                    out=po[:, :cols_blk],
                    lhsT=wv_sb[:],
                    rhs=vt_sb[:, i0 * WIN : i0 * WIN + cols_blk],
                    start=False, stop=True,
                )
                jc = j % OUT_CHUNK
                if jc == 0:
                    ot = op.tile(
                        [128, OUT_CHUNK * BLK_STEPS * WIN], bf16, tag="ot"
                    )
                    chunk_col0 = i0 * WIN
                nc.scalar.activation(
                    out=ot[:, jc * BLK_STEPS * WIN : jc * BLK_STEPS * WIN + cols_blk],
                    in_=po[:, :cols_blk],
                    func=mybir.ActivationFunctionType.Identity,
                    bias=b_sb[:, 0:1],
                    scale=1.0,
                )
                if jc == OUT_CHUNK - 1 or j == N_BLKS - 1:
                    chunk_cols = jc * BLK_STEPS * WIN + cols_blk
                    nc.scalar.dma_start(
                        outT_d[:, chunk_col0 : chunk_col0 + chunk_cols],
                        ot[:, :chunk_cols],
                    )
    _split_multi_waits(nc)
    return nc


# ---------------------------------------------------------------------------
# public entry point
# ---------------------------------------------------------------------------


def kernel(vdata, edata_a, edata_b, conn_a, conn_b, W, b, _trace=False):
    in_maps, sched, perms = _preprocess(
        vdata, edata_a, edata_b, conn_a, conn_b, W, b
    )
    nc = _NC_CACHE.get(sched)
    if nc is None:
        nc = _build(sched)
        _NC_CACHE[sched] = nc
    kwargs = {}
    if _trace:
        kwargs = dict(trace=True, trace_cores=[0])
    res = run_bass_kernel_spmd(
        nc, in_maps, core_ids=list(range(N_CORES)), **kwargs
    )

    out_full = np.empty((NTOT, D_OUT), dtype=np.float32)
    for c in range(N_CORES):
        outT = np.asarray(res.results[c]["outT"]).astype(np.float32)
        blocks = outT.reshape(D_OUT, WPC, WIN)
        base = c * NPC
        for i in range(WPC):
            w = perms[c][i]
            out_full[base + w * WIN : base + (w + 1) * WIN] = blocks[:, i, :].T
    out = out_full[:N_NODES]
    if _trace:
        return out, res
    return out
